# revision 11
# baseline (speedup 1.0000x reference)
"""Contrastive loss kernel for Trainium2 (8 NeuronCores, Bass/Tile).

Math: with L2-normalized embeddings, dist = 1 - sim and MARGIN = 2.0, the
negative branch relu(2 - dist) = 1 + sim is never clipped (|sim| <= 1), so

    pair_loss = (1+sim)^2 - 4*sim*[same]

Summing the strict upper triangle of the symmetric pair matrix:

    total = (B^2 + 2*||s||^2 + ||C||_F^2 - 4*sum_k ||g_k||^2)/2

where C = E^T E (DxD), g_k = sum_{key_i=k} e_i (128 key groups), s = sum_i e_i
(= column sum of G). Uses sum_ij sim^2 = tr((E^T E)^2) = ||C||_F^2. The
diagonal correction sum_i(1-||e_i||^2)^2 is O(B*eps^2) ~ 1e-10 and dropped.
This turns an O(B^2 D) problem into O(B D^2).

Distribution: measured on this fabric, an 8-core 384 KB AllReduce costs ~57us
-- far more than the O(B D^2) compute itself. So instead of row-sharding +
AllReduce (the hint), every core redundantly computes the full reduction from
the full embedding matrix (8 MB), which is fully independent per core: no
collective, no cross-core skew sensitivity. Per row-tile of 128 rows, the
concatenation F = [E_tile | onehot(keys_tile)] gives both C and G^T from two
accumulating fp32r matmuls: (F[:, :128])^T F and (F[:, 128:256])^T F.
"""

import sys

for _p in ("/opt/trn_rl_repo",):
    if _p not in sys.path:
        sys.path.insert(0, _p)

import numpy as np

import concourse.bass as bass
import concourse.bacc as bacc
import concourse.mybir as mybir
import concourse.tile as tile
from concourse.bass_utils import run_bass_kernel_spmd

B, D = 8192, 256
N_CORES = 8
NKEYS = 128
NUM_PAIRS = B * (B - 1) // 2
NT = B // 128            # 64 row-tiles of 128 rows
NCHUNK = 8               # DMA granularity: 8 chunks of 8 row-tiles (1 MB each)
TPC = NT // NCHUNK       # row-tiles per chunk
FW = D + NKEYS           # 384: [E | onehot] concat width

F32 = mybir.dt.float32
F32R = mybir.dt.float32r

_cache = {}


def _build():
    nc = bacc.Bacc(
        "TRN2",
        target_bir_lowering=False,
        debug=False,
        num_devices=N_CORES,
    )

    emb = nc.dram_tensor("emb", [B, D], F32, kind="ExternalInput").ap()
    # keysT[i, t] = order_keys[t*128 + i], as f32 (values < 128 exact)
    keysT = nc.dram_tensor("keysT", [128, NT], F32, kind="ExternalInput").ap()
    loss_out = nc.dram_tensor("loss", [1, 1], F32, kind="ExternalOutput").ap()

    # Row-tile views of emb, bitcast to f32r so the DMA writes f32r-typed
    # SBUF directly (PE consumes the unrounded bits; the .001% low-mantissa
    # difference vs a rounding copy is far inside the error budget).
    emb_r = emb.bitcast(F32R).rearrange("(t p) d -> t p d", t=NT, p=128)

    with tile.TileContext(nc) as tc:
        with (
            tc.tile_pool(name="const", bufs=1) as cpool,
            tc.tile_pool(name="work", bufs=3) as pool,
            tc.tile_pool(name="psum", bufs=1, space="PSUM") as psum,
        ):
            keys_sb = cpool.tile([128, NT], F32)
            nc.sync.dma_start(keys_sb[:], keysT[:])

            iota_sb = cpool.tile([128, NKEYS], F32)
            nc.gpsimd.iota(
                iota_sb[:],
                pattern=[[1, NKEYS]],
                base=0,
                channel_multiplier=0,
                allow_small_or_imprecise_dtypes=True,
            )

            ones_sb = cpool.tile([128, 1], F32)
            nc.vector.memset(ones_sb[:], 1.0)

            # p0 = [C00 | C01 | G^T rows 0:128]   (C row-half 0:128, full width)
            # p1 = [C11 | G^T rows 128:256]       (C01 block recovered by symmetry)
            p0 = psum.tile([128, FW], F32, name="p0")
            p1 = psum.tile([128, D], F32, name="p1")

            for t in range(NT):
                ft = pool.tile([128, FW], F32R, tag="ft", bufs=6)
                nc.sync.dma_start(ft[:, 0:D], emb_r[t])
                nc.vector.tensor_scalar(
                    ft[:, D:FW],
                    iota_sb[:],
                    keys_sb[:, t : t + 1],
                    None,
                    op0=mybir.AluOpType.is_equal,
                )
                first, last = t == 0, t == NT - 1
                nc.tensor.matmul(
                    p0[:], lhsT=ft[:, 0:128], rhs=ft[:], start=first, stop=last
                )
                nc.tensor.matmul(
                    p1[:], lhsT=ft[:, 128:256], rhs=ft[:, 128:FW], start=first, stop=last
                )

            # Move PSUM partials to SBUF for multi-read finals.
            r0 = pool.tile([128, FW], F32)
            nc.vector.tensor_copy(r0[:], p0[:])
            r1 = pool.tile([128, D], F32)
            nc.vector.tensor_copy(r1[:], p1[:])

            # Per-partition pieces. ||C||^2 = sum(C00^2) + 2*sum(C01^2) + sum(C11^2).
            aC00 = pool.tile([128, 1], F32)
            aC01 = pool.tile([128, 1], F32)
            aC11 = pool.tile([128, 1], F32)
            aG0 = pool.tile([128, 1], F32)
            aG1 = pool.tile([128, 1], F32)
            s0 = pool.tile([128, 1], F32)
            s1 = pool.tile([128, 1], F32)
            for acc, src in (
                (aC00, r0[:, 0:128]),
                (aC01, r0[:, 128:256]),
                (aC11, r1[:, 0:128]),
                (aG0, r0[:, D:FW]),
                (aG1, r1[:, 128:D]),
            ):
                sq = pool.tile([128, 128], F32, tag="sq", name=f"sq_{acc.name}")
                nc.vector.tensor_mul(sq[:], src, src)
                nc.vector.tensor_reduce(
                    acc[:], sq[:], axis=mybir.AxisListType.X, op=mybir.AluOpType.add
                )
            nc.vector.tensor_reduce(
                s0[:], r0[:, D:FW], axis=mybir.AxisListType.X, op=mybir.AluOpType.add
            )
            nc.vector.tensor_reduce(
                s1[:], r1[:, 128:D], axis=mybir.AxisListType.X, op=mybir.AluOpType.add
            )

            # comb2 = aC00 + 2*aC01 + aC11 - 4*(aG0+aG1) + 2*(s0^2+s1^2)
            aC01m = pool.tile([128, 1], F32)
            nc.vector.tensor_scalar_mul(aC01m[:], aC01[:], 2.0)
            tC = pool.tile([128, 1], F32)
            nc.vector.tensor_add(tC[:], aC00[:], aC01m[:])
            tC2 = pool.tile([128, 1], F32)
            nc.vector.tensor_add(tC2[:], tC[:], aC11[:])
            tG = pool.tile([128, 1], F32)
            nc.vector.tensor_add(tG[:], aG0[:], aG1[:])
            tGm = pool.tile([128, 1], F32)
            nc.vector.tensor_scalar_mul(tGm[:], tG[:], -4.0)
            ssq0 = pool.tile([128, 1], F32)
            nc.vector.tensor_mul(ssq0[:], s0[:], s0[:])
            ssq1 = pool.tile([128, 1], F32)
            nc.vector.tensor_mul(ssq1[:], s1[:], s1[:])
            tS = pool.tile([128, 1], F32)
            nc.vector.tensor_add(tS[:], ssq0[:], ssq1[:])
            tSm = pool.tile([128, 1], F32)
            nc.vector.tensor_scalar_mul(tSm[:], tS[:], 2.0)
            comb = pool.tile([128, 1], F32)
            nc.vector.tensor_add(comb[:], tC2[:], tGm[:])
            comb2 = pool.tile([128, 1], F32)
            nc.vector.tensor_add(comb2[:], comb[:], tSm[:])

            # t1 = sum_p comb2[p] via ones matmul, then affine to the loss.
            t1 = psum.tile([1, 1], F32, name="t1")
            nc.tensor.matmul(t1[:], lhsT=comb2[:], rhs=ones_sb[:], start=True, stop=True)
            t1m = pool.tile([1, 1], F32)
            nc.vector.tensor_scalar_mul(t1m[:], t1[:], 1.0 / (2.0 * NUM_PAIRS))
            res = pool.tile([1, 1], F32)
            nc.vector.tensor_scalar_add(
                res[:], t1m[:], float(B) * B / (2.0 * NUM_PAIRS)
            )
            nc.sync.dma_start(loss_out[:], res[:])

    nc.compile()
    return nc


def _get_nc():
    if "nc" not in _cache:
        _cache["nc"] = _build()
    return _cache["nc"]


def _in_maps(embeddings: np.ndarray, order_keys: np.ndarray):
    emb = np.ascontiguousarray(embeddings, dtype=np.float32)
    keys = np.ascontiguousarray(
        order_keys.astype(np.float32).reshape(NT, 128).T
    )
    return [{"emb": emb, "keysT": keys} for _ in range(N_CORES)]


def kernel(embeddings: np.ndarray, order_keys: np.ndarray) -> np.ndarray:
    nc = _get_nc()
    res = run_bass_kernel_spmd(nc, _in_maps(embeddings, order_keys), list(range(N_CORES)))
    return np.asarray(res.results[0]["loss"], dtype=np.float32).reshape(())
